# revision 1
# baseline (speedup 1.0000x reference)
"""GuidedAttentionLoss on 8 TRN2 cores — 2 width classes (512 / 256).

Rows (b, x) with il_b > 256 are processed at free-width 512; rows with
il_b <= 256 at width 256 (their att is zero beyond il anyway, so the
upper 256 columns would contribute exactly 0 — skip them).  Cuts padded
elements and DMA ~25% vs the single-class version.
"""
import numpy as np

N_CORES = 8
T_IN = 512
KEXP = -3.125

_cache = {}


def _build_program(T512, T256):
    import concourse.bacc as bacc
    import concourse.mybir as mybir
    import concourse.tile as tile

    F32 = mybir.dt.float32
    nc = bacc.Bacc("TRN2", target_bir_lowering=False, debug=False,
                   num_devices=1)
    T = T512 + T256
    A512 = nc.declare_dram_parameter("A512", [max(T512, 1) * 128, 512], F32,
                                     isOutput=False)
    A256 = nc.declare_dram_parameter("A256", [max(T256, 1) * 128, 256], F32,
                                     isOutput=False)
    yvp = nc.declare_dram_parameter("yv", [128, T_IN], F32, isOutput=False)
    scp = nc.declare_dram_parameter("sc", [128, T], F32, isOutput=False)
    bcp = nc.declare_dram_parameter("bc", [128, T], F32, isOutput=False)
    r1p = nc.declare_dram_parameter("r1", [128, T], F32, isOutput=True)
    r2p = nc.declare_dram_parameter("r2", [128, T], F32, isOutput=True)

    Sq = mybir.ActivationFunctionType.Square
    Ex = mybir.ActivationFunctionType.Exp
    sub = mybir.AluOpType.subtract
    mult = mybir.AluOpType.mult

    with tile.TileContext(nc) as tc:
        with tc.tile_pool(name="aux", bufs=1) as aux, \
             tc.tile_pool(name="p", bufs=4) as pa, \
             tc.tile_pool(name="pd", bufs=3) as pd, \
             tc.tile_pool(name="pe", bufs=3) as pe, \
             tc.tile_pool(name="pt", bufs=3) as pt, \
             tc.tile_pool(name="pq", bufs=3) as pq:
            yvt = aux.tile([128, T_IN], F32)
            nc.sync.dma_start(yvt[:], yvp[:])
            sct = aux.tile([128, T], F32)
            nc.sync.dma_start(sct[:], scp[:])
            bct = aux.tile([128, T], F32)
            nc.sync.dma_start(bct[:], bcp[:])
            r1t = aux.tile([128, T], F32)
            r2t = aux.tile([128, T], F32)

            def tile_body(t_i, W, src, row0):
                at = pa.tile([128, W], F32, tag="a")
                nc.sync.dma_start(at[:], src[row0:row0 + 128, :])
                d = pd.tile([128, W], F32, tag="d")
                nc.scalar.activation(d[:], yvt[:, :W], Sq,
                                     bias=bct[:, t_i:t_i + 1],
                                     scale=sct[:, t_i:t_i + 1])
                e = pe.tile([128, W], F32, tag="e")
                nc.scalar.activation(e[:], d[:], Ex, scale=KEXP)
                t = pt.tile([128, W], F32, tag="t")
                nc.vector.scalar_tensor_tensor(
                    t[:], e[:], 1.0, at[:], sub, mult,
                    accum_out=r1t[:, t_i:t_i + 1])
                tsq = pq.tile([128, W], F32, tag="q")
                if t_i % 7 < 5:
                    nc.vector.scalar_tensor_tensor(
                        tsq[:], t[:], 0.0, t[:], sub, mult,
                        accum_out=r2t[:, t_i:t_i + 1])
                else:
                    nc.scalar.activation(tsq[:], t[:], Sq,
                                         accum_out=r2t[:, t_i:t_i + 1])

            for i in range(T512):
                tile_body(i, 512, A512, i * 128)
            for i in range(T256):
                tile_body(T512 + i, 256, A256, i * 128)

            nc.sync.dma_start(r1p[:], r1t[:])
            nc.sync.dma_start(r2p[:], r2t[:])
    nc.compile()
    return nc


def _pack_class(att_ws, il, ol, rows_b, rows_x, n_core_rows, W):
    """rows_b/rows_x: global row lists for this class. Returns per-core
    A arrays + per-core (row_b) map, padded to n_core_rows rows/core."""
    B = att_ws.shape[0]
    nrows = len(rows_b)
    A = np.zeros((N_CORES, n_core_rows, W), np.float32)
    mb = np.full((N_CORES, n_core_rows), -1, np.int64)
    mx = np.zeros((N_CORES, n_core_rows), np.int64)
    # contiguous block split
    base = 0
    for c in range(N_CORES):
        take = min(n_core_rows, max(0, nrows - base))
        if take:
            rb = rows_b[base:base + take]
            rx = rows_x[base:base + take]
            mb[c, :take] = rb
            mx[c, :take] = rx
        base += take
    # fill A row-by-row grouped by sample for speed
    for c in range(N_CORES):
        rb = mb[c]
        rx = mx[c]
        for b in np.unique(rb):
            if b < 0:
                continue
            sel = rb == b
            i = min(int(il[b]), W)
            A[c, sel, :i] = att_ws[b, rx[sel], :i]
    return A, mb, mx


def kernel(att_ws, ilens, olens, _trace=False, _tracedir=None):
    from concourse.bass_utils import run_bass_kernel_spmd

    att_ws = np.asarray(att_ws)
    il = np.asarray(ilens).astype(np.int64)
    ol = np.asarray(olens).astype(np.int64)
    B, T_out, T_in = att_ws.shape

    big = il > 256          # class-512 samples
    rb_l, rx_l = [], []
    for cls in (True, False):
        sel = np.nonzero(big == cls)[0]
        rb = np.repeat(sel, ol[sel])
        rx = np.concatenate([np.arange(int(ol[b])) for b in sel]) \
            if len(sel) else np.zeros(0, np.int64)
        rb_l.append(rb)
        rx_l.append(rx)

    R512, R256 = len(rb_l[0]), len(rb_l[1])
    T512 = -(-(-(-R512 // N_CORES)) // 128) if R512 else 0
    T256 = -(-(-(-R256 // N_CORES)) // 128) if R256 else 0
    T = T512 + T256

    A5, mb5, mx5 = _pack_class(att_ws, il, ol, rb_l[0], rx_l[0],
                               T512 * 128 if T512 else 0, 512)
    A2, mb2, mx2 = _pack_class(att_ws, il, ol, rb_l[1], rx_l[1],
                               T256 * 128 if T256 else 0, 256)

    ilf = il.astype(np.float32)
    olf = ol.astype(np.float64)
    yv = np.broadcast_to(np.arange(T_IN, dtype=np.float32),
                         (128, T_IN)).copy()

    in_maps = []
    maps = []
    for c in range(N_CORES):
        mb = np.concatenate([mb5[c] if T512 else np.zeros(0, np.int64),
                             mb2[c] if T256 else np.zeros(0, np.int64)])
        mx = np.concatenate([mx5[c] if T512 else np.zeros(0, np.int64),
                             mx2[c] if T256 else np.zeros(0, np.int64)])
        v = mb >= 0
        sc = np.zeros(len(mb), np.float32)
        bc = np.zeros(len(mb), np.float32)
        sc[v] = 1.0 / ilf[mb[v]]
        bc[v] = -(mx[v] / olf[mb[v]]).astype(np.float32)
        in_maps.append({
            "A512": A5[c] if T512 else np.zeros((128, 512), np.float32),
            "A256": A2[c] if T256 else np.zeros((128, 256), np.float32),
            "yv": yv,
            "sc": np.ascontiguousarray(sc.reshape(T, 128).T),
            "bc": np.ascontiguousarray(bc.reshape(T, 128).T),
        })
        maps.append(mb)

    key = (T512, T256)
    if key not in _cache:
        _cache[key] = _build_program(T512, T256)
    nc = _cache[key]
    kw = {}
    if _trace:
        kw = dict(trace=True, tmpdir=_tracedir)
    res = run_bass_kernel_spmd(nc, in_maps, list(range(N_CORES)), **kw)
    kernel._last_exec_ns = getattr(res, "exec_time_ns", None)

    sum1 = np.zeros(B, np.float64)
    sum2 = np.zeros(B, np.float64)
    for c in range(N_CORES):
        r1 = np.asarray(res.results[c]["r1"], np.float64).T.reshape(-1)
        r2 = np.asarray(res.results[c]["r2"], np.float64).T.reshape(-1)
        mb = maps[c]
        m = mb >= 0
        np.add.at(sum1, mb[m], r1[m])
        np.add.at(sum2, mb[m], r2[m])

    l1 = (-sum1 / olf).astype(np.float32)
    l2 = (sum2 / olf).astype(np.float32)
    return (l1, l2)



# revision 2
# speedup vs baseline: 1.0487x; 1.0487x over previous
"""GuidedAttentionLoss on 8 TRN2 cores — transposed-layout PE/Act/DVE/Pool
pipeline.

Layout: partition dim = input tokens y, free dim = output frames x.
Per 128-partition "unit" (stack of (sample, y-block, x-piece) items):
  PE:   arg[p,x] = A_p*x + C_p*(x^2/4096)   (K=2 matmul, fp16)
  Act:  e = Exp(arg + bias_p)               (bias_p = -3.125*(y/il)^2, f32)
  DVE/Pool: t = (e-1)*a  with accum -> r1 column   (a in bf16)
  DVE:  q = t*t (bf16, 2x mode)
  PE:   strip slot += ind.T @ q  (segmented per-item column sums, 64-wide)
Host packs inputs (transpose/shard/cast) and does the final tiny gather.
"""
import numpy as np
import ml_dtypes

N_CORES = 8
SIGMA = 0.4
KC = 1.0 / (2 * SIGMA * SIGMA)  # 3.125
LMAX = 1536        # x-piece cap (arg psum tile: 3 banks)
MCAP = 8           # items per unit cap (ind columns / strip slot rows)
XMAX = 2048
LQUANT = 64

_cache = {}


# --------------------------------------------------------------- packing ---
def _pack(il, ol):
    """Build global units. Returns list of units; each unit:
    (L, items) with items = list of (b, k, xoff, xlen, h, p0)."""
    items = []
    B = len(il)
    for b in range(B):
        nb = (int(il[b]) + 127) // 128
        for k in range(nb):
            h = min(int(il[b]) - 128 * k, 128)
            xoff = 0
            while xoff < int(ol[b]):
                xlen = min(LMAX, int(ol[b]) - xoff)
                items.append((b, k, xoff, xlen, h))
                xoff += xlen
    # first-fit-decreasing on partition height, items in xlen-desc order so
    # a unit's L (max xlen = first item's xlen) never grows when stacking
    items.sort(key=lambda t: -t[3])
    units = []       # list of item lists
    room = []        # partition rows left per unit
    for it in items:
        h = it[4]
        for i, r in enumerate(room):
            if r >= h and len(units[i]) < MCAP:
                units[i].append(it)
                room[i] -= h
                break
        else:
            units.append([it])
            room.append(128 - h)
    out = []
    for u in units:
        L = -(-max(it[3] for it in u) // LQUANT) * LQUANT
        p0 = 0
        its = []
        for (b, k, xoff, xlen, h) in u:
            its.append((b, k, xoff, xlen, h, p0))
            p0 += h
        out.append((L, its))
    # sort units by L desc for striping
    out.sort(key=lambda t: -t[0])
    return out


def _stripe(units):
    """Assign global unit j -> (core j%8, stripe rank j//8), then permute
    stripe emission order: a couple of small stripes first (fast pipeline
    fill), the big ones in the middle, small again at the end (fast drain).
    Returns (U, widths_in_emission_order, ranks) with ranks[slot] = stripe
    rank (desc-width index) emitted at that slot."""
    U = (len(units) + N_CORES - 1) // N_CORES
    desc = [units[s * N_CORES][0] for s in range(U)]  # desc widths
    order = []
    if U > 3:
        order = [U - 1, U - 3] + list(range(U - 3)) + [U - 2]
    else:
        order = list(range(U))
    widths = [desc[r] for r in order]
    return U, widths, tuple(order)


def _engine_plan(widths):
    """Per stripe: 'v' (DVE) or 'g' (Pool) for the q = t*t pass, balancing
    measured busy rates (Pool TT costs 0.833 ns/col in the cost model; DVE
    TT 0.52 but DVE already runs every stt at 1.04)."""
    dve = sum(w * 1.0417 for w in widths) + 1300.0  # stt passes + copies
    pool = 0.0
    plan = []
    for w in widths:
        if dve + w * 0.5208 <= pool + w * 0.8333 + 95.0:
            plan.append("v")
            dve += w * 0.5208
        else:
            plan.append("g")
            pool += w * 0.8333 + 95.0
    return tuple(plan)


# --------------------------------------------------------------- program ---
def _build_program(widths, plan, nchunks_dma):
    import concourse.bacc as bacc
    import concourse.mybir as mybir
    import concourse.tile as tile

    F32 = mybir.dt.float32
    BF16 = mybir.dt.bfloat16
    FP16 = mybir.dt.float16
    Exp = mybir.ActivationFunctionType.Exp
    sub = mybir.AluOpType.subtract
    mult = mybir.AluOpType.mult

    U = len(widths)
    F = sum(widths)
    G = (U + 31) // 32  # strip generations

    # group stripes into DMA chunks: small first chunks so compute starts
    # early, growing to ~6K cols (12KB/partition, under the 64KB desc cap)
    chunks = []  # (u_start, u_end, col_off, width)
    u0, c0 = 0, 0
    targets = [512, 1536, 3072]  # then 6144
    for u in range(U + 1):
        w = sum(widths[u0:u])
        tgt = targets[len(chunks)] if len(chunks) < len(targets) else 6144
        if u == U or (w >= tgt and u > u0):
            if u > u0:
                chunks.append((u0, u, c0, w))
                c0 += w
                u0 = u

    nc = bacc.Bacc("TRN2", target_bir_lowering=False, debug=False,
                   num_devices=1)
    A = nc.declare_dram_parameter("A", [128, F], BF16, isOutput=False)
    # RC = [x-ramp rows (x, x^2/4096, 1) | per-unit (A,C,bias) coeff
    # columns], one fp16 aux load; bias rides the matmul as a K=3 row
    RC = nc.declare_dram_parameter("RC", [3, XMAX + 128 * U], FP16,
                                   isOutput=False)
    IND = nc.declare_dram_parameter("IND", [128, MCAP * U], BF16,
                                    isOutput=False)
    R1 = nc.declare_dram_parameter("R1", [128, U], F32, isOutput=True)
    STR = nc.declare_dram_parameter("STR", [128, 512 * G], F32, isOutput=True)

    with tile.TileContext(nc) as tc:
        with tc.tile_pool(name="aux", bufs=1) as aux, \
             tc.tile_pool(name="pa", bufs=1) as pa, \
             tc.tile_pool(name="pe_", bufs=4) as pev, \
             tc.tile_pool(name="pt", bufs=4) as pt, \
             tc.tile_pool(name="pq", bufs=4) as pq, \
             tc.tile_pool(name="psarg", bufs=2, space="PSUM") as psarg, \
             tc.tile_pool(name="psstr", bufs=2, space="PSUM") as psstr:
            # issue order matters: rc (PE needs it first), then the first
            # small a-chunk (stt), then bias/ind, then remaining chunks
            rc = aux.tile([3, XMAX + 128 * U], FP16)
            nc.sync.dma_start(rc[:], RC[:])
            ramp = rc[:, :XMAX]
            ind = aux.tile([128, MCAP * U], BF16)
            r1 = aux.tile([128, U], F32)

            # chunked input loads (separate tiles so compute can start early)
            atiles = []
            for ci, (us, ue, coff, w) in enumerate(chunks):
                t_ = pa.tile([128, w], BF16, tag=f"a{us}")
                nc.sync.dma_start(t_[:], A[:, coff:coff + w])
                atiles.append((us, ue, coff, t_))
                if ci == 0:
                    nc.sync.dma_start(ind[:], IND[:])

            def a_slice(u, off_u, L):
                for (us, ue, coff, t_) in atiles:
                    if us <= u < ue:
                        return t_[:, off_u - coff:off_u - coff + L]
                raise AssertionError

            strip = None
            stage = None
            off = 0
            # xoff per stripe is geometry: ramp slice start. xoff is the
            # SAME for all items in a unit? No - items have different xoff.
            # arg uses absolute x: ramp slice must match each item's x range.
            # => all items in a unit must share the same x-window [xw, xw+L).
            # Handled in packing: xoff passed via widths-aligned plan...
            # (see _pack2: units carry xw; here xw comes in via plan tuple)
            for u in range(U):
                L = widths[u]
                g, r = u // 32, u % 32
                sr, sc = 32 * (r % 4), 64 * (r // 4)
                if r == 0:
                    strip = psstr.tile([128, 512], F32, tag="strip")
                    nc.vector.memset(strip[:], 0.0)
                arg = psarg.tile([128, LMAX], F32, tag="arg")
                for c in range(0, L, 512):
                    cw = min(512, L - c)
                    nc.tensor.matmul(
                        arg[:, c:c + cw],
                        rc[:, XMAX + 128 * u:XMAX + 128 * u + 128],
                        ramp[:, c:c + cw],
                        start=True, stop=True)
                et = pev.tile([128, L], BF16, tag="e")
                nc.scalar.activation(et[:], arg[:, :L], Exp, scale=1.0)
                tt = pt.tile([128, L], BF16, tag="t")
                nc.vector.scalar_tensor_tensor(
                    tt[:], et[:], 1.0, a_slice(u, off, L), sub, mult,
                    accum_out=r1[:, u:u + 1])
                qt = pq.tile([128, L], BF16, tag="q")
                qeng = nc.vector if plan[u] == "v" else nc.gpsimd
                qeng.tensor_tensor(qt[:], tt[:], tt[:], mult)
                nch = (L + 63) // 64
                for ci, c in enumerate(range(0, L, 64)):
                    cw = min(64, L - c)
                    nc.tensor.matmul(
                        strip[sr:sr + MCAP, sc:sc + cw],
                        ind[:, MCAP * u:MCAP * u + MCAP],
                        qt[:, c:c + cw],
                        start=(ci == 0), stop=(ci == nch - 1),
                        tile_position=(0, sr))
                last = u == U - 1
                if (r % 8 == 7 and not last) or (last and r % 8 != 7):
                    # flush completed 128-col block(s) of this generation
                    lo = 128 * (r // 8)
                    hi = lo + 128 if not last else 64 * (r // 4) + 64
                    sh = pev.tile([128, 512], F32, tag="stg")
                    nc.vector.tensor_copy(sh[:, lo:hi], strip[:, lo:hi])
                    nc.sync.dma_start(STR[:, 512 * g + lo:512 * g + hi],
                                      sh[:, lo:hi])
                elif last:
                    lo = 128 * (r // 8)
                    hi = 64 * (r // 4) + 64
                    sh = pev.tile([128, 512], F32, tag="stg")
                    nc.vector.tensor_copy(sh[:, lo:hi], strip[:, lo:hi])
                    nc.sync.dma_start(STR[:, 512 * g + lo:512 * g + hi],
                                      sh[:, lo:hi])
                if u == (3 * U) // 4:
                    usplit = u + 1
                    nc.sync.dma_start(R1[:, :usplit], r1[:, :usplit])
                off += L

            nc.sync.dma_start(R1[:, usplit:], r1[:, usplit:])
    nc.compile()
    return nc


# ---------------------------------------------------------------- kernel ---
def kernel(att_ws, ilens, olens, _trace=False, _tracedir=None):
    from concourse.bass_utils import run_bass_kernel_spmd

    att_ws = np.asarray(att_ws)
    il = np.asarray(ilens).astype(np.int64)
    ol = np.asarray(olens).astype(np.int64)
    B, T_out, T_in = att_ws.shape

    units = _pack(il, ol)
    U, widths, ranks = _stripe(units)
    plan = _engine_plan(widths)
    G = (U + 31) // 32
    F = sum(widths)

    # x-window consistency: items within a global unit may have different
    # xoff. The program indexes ramp[:, 0:L] (x starting at 0). For items
    # with xoff > 0 we instead fold the shift into per-partition constants:
    # arg(x') = A_p*(x'+xoff) + C_p*(x'+xoff)^2/..  -- quadratic in x' with
    # modified A'_p, C'_p (C unchanged), bias absorbs constant:
    #   A'_p = A_p + 2*xoff*Cq_p ; bias' += A_p*xoff + Cq_p*xoff^2
    # where Cq_p = -KC/ol^2 (true x^2 coeff; C_p = 4096*Cq_p).
    ilf = il.astype(np.float64)
    olf = ol.astype(np.float64)

    offs = np.concatenate([[0], np.cumsum(widths)]).astype(np.int64)
    in_maps = []
    meta = []  # per core: list of (j, item list)
    x = np.arange(XMAX, dtype=np.float64)
    RAMP = np.stack([x, x * x / 4096.0, np.ones(XMAX)])

    for c in range(N_CORES):
        Ar = np.zeros((128, F), np.float32)
        AC = np.zeros((3, 128 * U), np.float32)
        IND = np.zeros((128, MCAP * U), np.float32)
        mymeta = []
        for j in range(U):
            gidx = ranks[j] * N_CORES + c
            if gidx >= len(units):
                continue
            L, its = units[gidx]
            assert L <= widths[j]
            for i, (b, k, xoff, xlen, h, p0) in enumerate(its):
                yv = (128 * k + np.arange(h)).astype(np.float64)
                z = yv / ilf[b]
                Aq = 2.0 * KC * z / olf[b]          # x coeff
                Cq = -KC / (olf[b] ** 2)            # x^2 coeff
                Ap = Aq + 2.0 * xoff * Cq
                biasp = -KC * z * z + Aq * xoff + Cq * xoff * xoff
                AC[0, 128 * j + p0:128 * j + p0 + h] = Ap
                AC[1, 128 * j + p0:128 * j + p0 + h] = 4096.0 * Cq
                AC[2, 128 * j + p0:128 * j + p0 + h] = biasp
                IND[p0:p0 + h, MCAP * j + i] = 1.0
                Ar[p0:p0 + h, offs[j]:offs[j] + xlen] = \
                    att_ws[b, xoff:xoff + xlen, 128 * k:128 * k + h].T
                mymeta.append((j, i, b, p0, h))
        in_maps.append({
            "A": Ar.astype(ml_dtypes.bfloat16),
            "RC": np.concatenate([RAMP, AC], axis=1).astype(np.float16),
            "IND": IND.astype(ml_dtypes.bfloat16),
        })
        meta.append(mymeta)

    key = (tuple(widths), plan)
    if key not in _cache:
        _cache[key] = _build_program(list(widths), plan, 8)
    nc = _cache[key]
    kw = {}
    if _trace:
        kw = dict(trace=True, tmpdir=_tracedir)
    res = run_bass_kernel_spmd(nc, in_maps, list(range(N_CORES)), **kw)

    sum1 = np.zeros(B, np.float64)
    sum2 = np.zeros(B, np.float64)
    for c in range(N_CORES):
        r1 = np.asarray(res.results[c]["R1"], np.float64)
        st = np.asarray(res.results[c]["STR"], np.float64)
        for (j, i, b, p0, h) in meta[c]:
            sum1[b] += r1[p0:p0 + h, j].sum()
            g, r = j // 32, j % 32
            sr, sc = 32 * (r % 4), 64 * (r // 4)
            sum2[b] += st[sr + i, 512 * g + sc:512 * g + sc + 64].sum()

    l1 = (-sum1 / olf).astype(np.float32)
    l2 = (sum2 / olf).astype(np.float32)
    return (l1, l2)


# revision 4
# speedup vs baseline: 1.0756x; 1.0257x over previous
"""GuidedAttentionLoss on 8 TRN2 cores — transposed-layout pipeline using
all five engines.

Layout: partition dim = input tokens y, free dim = output frames x
(att_ws transposed per sample). Work is packed into 128-partition "units"
(stacks of (sample, y-block, x-piece) items, first-fit-decreasing on
height), striped round-robin across the 8 cores with shared geometry.

Per unit (L = padded x-width):
  PE:   arg[p,x] = A_p*x + C_p*(x^2/4096) + bias_p  (K=3 fp16 matmul
        against a shared [x, x^2/4096, 1] ramp; per-partition coeffs fold
        the full guided-attention quadratic -3.125*(y/il - x/ol)^2,
        including x-piece offsets)
  Act:  e = Exp(arg)                                 (PSUM -> SBUF bf16)
  DVE:  t = (e - 1)*a   scalar_tensor_tensor, accum_out -> r1[:, unit]
  Pool: q = t*t         (gpsimd tensor_tensor, bf16)
  PE:   strip[slot] += ind.T @ q   (64-wide accumulating indicator
        matmuls -> per-item partial column sums in a PSUM strip bank)
Strips are staged to SBUF and DMA'd out in 128-col blocks as they
complete; the host does the final tiny per-item gather (sum of 64 f32 +
per-partition r1 sums) in f64.

Input att_ws is cast to bf16 on the host (rel tol is 2e-2; measured total
error ~6e-4). HW exec time (CoreSim cost model): ~30.3 us vs 81.4 us for
the previous width-class row-layout kernel.
"""
import numpy as np
import ml_dtypes

N_CORES = 8
SIGMA = 0.4
KC = 1.0 / (2 * SIGMA * SIGMA)  # 3.125
LMAX = 1536        # x-piece cap (arg psum tile: 3 banks)
MCAP = 8           # items per unit cap (ind columns / strip slot rows)
XMAX = 2048
LQUANT = 32

_cache = {}


# --------------------------------------------------------------- packing ---
def _pack(il, ol):
    """Build global units. Returns list of units; each unit:
    (L, items) with items = list of (b, k, xoff, xlen, h, p0)."""
    items = []
    B = len(il)
    for b in range(B):
        nb = (int(il[b]) + 127) // 128
        for k in range(nb):
            h = min(int(il[b]) - 128 * k, 128)
            xoff = 0
            while xoff < int(ol[b]):
                xlen = min(LMAX, int(ol[b]) - xoff)
                items.append((b, k, xoff, xlen, h))
                xoff += xlen
    # first-fit-decreasing on partition height, items in xlen-desc order so
    # a unit's L (max xlen = first item's xlen) never grows when stacking
    items.sort(key=lambda t: -t[3])
    units = []       # list of item lists
    room = []        # partition rows left per unit
    for it in items:
        h = it[4]
        for i, r in enumerate(room):
            if r >= h and len(units[i]) < MCAP:
                units[i].append(it)
                room[i] -= h
                break
        else:
            units.append([it])
            room.append(128 - h)
    out = []
    for u in units:
        L = -(-max(it[3] for it in u) // LQUANT) * LQUANT
        p0 = 0
        its = []
        for (b, k, xoff, xlen, h) in u:
            its.append((b, k, xoff, xlen, h, p0))
            p0 += h
        out.append((L, its))
    # sort units by L desc for striping
    out.sort(key=lambda t: -t[0])
    return out


def _stripe(units):
    """Assign global unit j -> (core j%8, stripe rank j//8), then permute
    stripe emission order: a couple of small stripes first (fast pipeline
    fill), the big ones in the middle, small again at the end (fast drain).
    Returns (U, widths_in_emission_order, ranks) with ranks[slot] = stripe
    rank (desc-width index) emitted at that slot."""
    U = (len(units) + N_CORES - 1) // N_CORES
    desc = [units[s * N_CORES][0] for s in range(U)]  # desc widths
    order = []
    if U > 3:
        order = [U - 1, U - 3] + list(range(U - 3)) + [U - 2]
    else:
        order = list(range(U))
    widths = [desc[r] for r in order]
    return U, widths, tuple(order)


def _engine_plan(widths):
    """Per stripe: 'v' (DVE) or 'g' (Pool) for the q = t*t pass, balancing
    measured busy rates (Pool TT costs 0.833 ns/col in the cost model; DVE
    TT 0.52 but DVE already runs every stt at 1.04)."""
    dve = sum(w * 1.0417 for w in widths) + 1300.0  # stt passes + copies
    pool = 0.0
    plan = []
    for w in widths:
        if dve + w * 0.5208 + 400.0 < pool + w * 0.8333 + 95.0:
            plan.append("v")
            dve += w * 0.5208
        else:
            plan.append("g")
            pool += w * 0.8333 + 95.0
    return tuple(plan)


# --------------------------------------------------------------- program ---
def _build_program(widths, plan, nchunks_dma):
    import concourse.bacc as bacc
    import concourse.mybir as mybir
    import concourse.tile as tile

    F32 = mybir.dt.float32
    BF16 = mybir.dt.bfloat16
    FP16 = mybir.dt.float16
    Exp = mybir.ActivationFunctionType.Exp
    sub = mybir.AluOpType.subtract
    mult = mybir.AluOpType.mult

    U = len(widths)
    F = sum(widths)
    G = (U + 31) // 32  # strip generations

    # group stripes into DMA chunks: small first chunks so compute starts
    # early, growing to ~6K cols (12KB/partition, under the 64KB desc cap)
    chunks = []  # (u_start, u_end, col_off, width)
    u0, c0 = 0, 0
    targets = [512, 1536, 3072]  # then 6144
    for u in range(U + 1):
        w = sum(widths[u0:u])
        tgt = targets[len(chunks)] if len(chunks) < len(targets) else 6144
        if u == U or (w >= tgt and u > u0):
            if u > u0:
                chunks.append((u0, u, c0, w))
                c0 += w
                u0 = u

    nc = bacc.Bacc("TRN2", target_bir_lowering=False, debug=False,
                   num_devices=1)
    A = nc.declare_dram_parameter("A", [128, F], BF16, isOutput=False)
    # RC = [x-ramp rows (x, x^2/4096, 1) | per-unit (A,C,bias) coeff
    # columns], one fp16 aux load; bias rides the matmul as a K=3 row
    RC = nc.declare_dram_parameter("RC", [3, XMAX + 128 * U], FP16,
                                   isOutput=False)
    IND = nc.declare_dram_parameter("IND", [128, MCAP * U], BF16,
                                    isOutput=False)
    R1 = nc.declare_dram_parameter("R1", [128, U], F32, isOutput=True)
    STR = nc.declare_dram_parameter("STR", [128, 512 * G], F32, isOutput=True)

    with tile.TileContext(nc) as tc:
        with tc.tile_pool(name="aux", bufs=1) as aux, \
             tc.tile_pool(name="pa", bufs=1) as pa, \
             tc.tile_pool(name="pe_", bufs=4) as pev, \
             tc.tile_pool(name="pt", bufs=4) as pt, \
             tc.tile_pool(name="pq", bufs=4) as pq, \
             tc.tile_pool(name="psarg", bufs=2, space="PSUM") as psarg, \
             tc.tile_pool(name="psstr", bufs=2, space="PSUM") as psstr:
            # issue order matters: rc (PE needs it first), then the first
            # small a-chunk (stt), then bias/ind, then remaining chunks
            warm = aux.tile([128, 1], F32)
            nc.vector.memset(warm[:], 0.0)
            warm2 = aux.tile([128, 1], F32)
            nc.scalar.activation(warm2[:], warm[:], Exp, scale=1.0)
            rc = aux.tile([3, XMAX + 128 * U], FP16)
            nc.sync.dma_start(rc[:], RC[:])
            ramp = rc[:, :XMAX]
            ind = aux.tile([128, MCAP * U], BF16)
            r1 = aux.tile([128, U], F32)

            # chunked input loads (separate tiles so compute can start early)
            atiles = []
            for ci, (us, ue, coff, w) in enumerate(chunks):
                t_ = pa.tile([128, w], BF16, tag=f"a{us}")
                nc.sync.dma_start(t_[:], A[:, coff:coff + w])
                atiles.append((us, ue, coff, t_))
                if ci == 0:
                    nc.sync.dma_start(ind[:], IND[:])

            def a_slice(u, off_u, L):
                for (us, ue, coff, t_) in atiles:
                    if us <= u < ue:
                        return t_[:, off_u - coff:off_u - coff + L]
                raise AssertionError

            strip = None
            stage = None
            off = 0
            # xoff per stripe is geometry: ramp slice start. xoff is the
            # SAME for all items in a unit? No - items have different xoff.
            # arg uses absolute x: ramp slice must match each item's x range.
            # => all items in a unit must share the same x-window [xw, xw+L).
            # Handled in packing: xoff passed via widths-aligned plan...
            # (see _pack2: units carry xw; here xw comes in via plan tuple)
            # pair consecutive <=768-wide units: both args share one psum
            # tile (B at column 768) and a single Act Exp covers both
            pair_first = {}
            pair_second = {}
            # (pairing disabled: measured slower than per-unit Act)

            et_pair = None
            for u in range(U):
                L = widths[u]
                g, r = u // 32, u % 32
                sr, sc = 32 * (r % 4), 64 * (r // 4)
                if r == 0:
                    strip = psstr.tile([128, 512], F32, tag="strip")
                    nc.vector.memset(strip[:], 0.0)
                if u in pair_second:
                    et = et_pair[:, 768:768 + L]
                elif u in pair_first:
                    ub = pair_first[u]
                    lb = widths[ub]
                    arg = psarg.tile([128, LMAX], F32, tag="arg")
                    # A's chunks extended to the full 768 cols: the gap
                    # [L_A, 768) computes real arg values (always <= 0, so
                    # exp <= 1) that nothing reads — keeps psum initialized
                    for uu, base, ext in ((u, 0, 768), (ub, 768, lb)):
                        for c in range(0, ext, 512):
                            cw = min(512, ext - c)
                            bb = base + c
                            cw = min(cw, 512 - (bb % 512)) if bb % 512 \
                                else cw
                            nc.tensor.matmul(
                                arg[:, bb:bb + cw],
                                rc[:, XMAX + 128 * uu:XMAX + 128 * uu + 128],
                                ramp[:, c:c + cw],
                                start=True, stop=True)
                            if cw < min(512, ext - c):
                                c2 = c + cw
                                cw2 = min(512, ext - c) - cw
                                nc.tensor.matmul(
                                    arg[:, base + c2:base + c2 + cw2],
                                    rc[:, XMAX + 128 * uu:
                                        XMAX + 128 * uu + 128],
                                    ramp[:, c2:c2 + cw2],
                                    start=True, stop=True)
                    et_pair = pev.tile([128, 768 + lb], BF16, tag="e")
                    nc.scalar.activation(et_pair[:], arg[:, :768 + lb],
                                         Exp, scale=1.0)
                    et = et_pair[:, :L]
                else:
                    arg = psarg.tile([128, LMAX], F32, tag="arg")
                    for c in range(0, L, 512):
                        cw = min(512, L - c)
                        nc.tensor.matmul(
                            arg[:, c:c + cw],
                            rc[:, XMAX + 128 * u:XMAX + 128 * u + 128],
                            ramp[:, c:c + cw],
                            start=True, stop=True)
                    et_full = pev.tile([128, L], BF16, tag="e")
                    nc.scalar.activation(et_full[:], arg[:, :L], Exp,
                                         scale=1.0)
                    et = et_full[:]
                tt = pt.tile([128, L], BF16, tag="t")
                nc.vector.scalar_tensor_tensor(
                    tt[:], et[:], 1.0, a_slice(u, off, L), sub, mult,
                    accum_out=r1[:, u:u + 1])
                qt = pq.tile([128, L], BF16, tag="q")
                qeng = nc.vector if plan[u] == "v" else nc.gpsimd
                qeng.tensor_tensor(qt[:], tt[:], tt[:], mult)
                nch = (L + 63) // 64
                for ci, c in enumerate(range(0, L, 64)):
                    cw = min(64, L - c)
                    nc.tensor.matmul(
                        strip[sr:sr + MCAP, sc:sc + cw],
                        ind[:, MCAP * u:MCAP * u + MCAP],
                        qt[:, c:c + cw],
                        start=(ci == 0), stop=(ci == nch - 1),
                        tile_position=(0, sr))
                last = u == U - 1
                if (r % 8 == 7 and not last) or (last and r % 8 != 7):
                    # flush completed 128-col block(s) of this generation
                    lo = 128 * (r // 8)
                    hi = lo + 128 if not last else 64 * (r // 4) + 64
                    sh = pev.tile([128, 512], F32, tag="stg")
                    nc.vector.tensor_copy(sh[:, lo:hi], strip[:, lo:hi])
                    nc.sync.dma_start(STR[:, 512 * g + lo:512 * g + hi],
                                      sh[:, lo:hi])
                elif last:
                    lo = 128 * (r // 8)
                    hi = 64 * (r // 4) + 64
                    sh = pev.tile([128, 512], F32, tag="stg")
                    nc.vector.tensor_copy(sh[:, lo:hi], strip[:, lo:hi])
                    nc.sync.dma_start(STR[:, 512 * g + lo:512 * g + hi],
                                      sh[:, lo:hi])
                if u == (3 * U) // 4:
                    usplit = u + 1
                    nc.sync.dma_start(R1[:, :usplit], r1[:, :usplit])
                off += L

            nc.scalar.dma_start(R1[:, usplit:], r1[:, usplit:])
    nc.compile()
    return nc


# ---------------------------------------------------------------- kernel ---
def kernel(att_ws, ilens, olens, _trace=False, _tracedir=None):
    from concourse.bass_utils import run_bass_kernel_spmd

    att_ws = np.asarray(att_ws)
    il = np.asarray(ilens).astype(np.int64)
    ol = np.asarray(olens).astype(np.int64)
    B, T_out, T_in = att_ws.shape

    units = _pack(il, ol)
    U, widths, ranks = _stripe(units)
    plan = _engine_plan(widths)
    G = (U + 31) // 32
    F = sum(widths)

    # x-window consistency: items within a global unit may have different
    # xoff. The program indexes ramp[:, 0:L] (x starting at 0). For items
    # with xoff > 0 we instead fold the shift into per-partition constants:
    # arg(x') = A_p*(x'+xoff) + C_p*(x'+xoff)^2/..  -- quadratic in x' with
    # modified A'_p, C'_p (C unchanged), bias absorbs constant:
    #   A'_p = A_p + 2*xoff*Cq_p ; bias' += A_p*xoff + Cq_p*xoff^2
    # where Cq_p = -KC/ol^2 (true x^2 coeff; C_p = 4096*Cq_p).
    ilf = il.astype(np.float64)
    olf = ol.astype(np.float64)

    offs = np.concatenate([[0], np.cumsum(widths)]).astype(np.int64)
    in_maps = []
    meta = []  # per core: list of (j, item list)
    x = np.arange(XMAX, dtype=np.float64)
    RAMP = np.stack([x, x * x / 4096.0, np.ones(XMAX)])

    for c in range(N_CORES):
        Ar = np.zeros((128, F), np.float32)
        AC = np.zeros((3, 128 * U), np.float32)
        IND = np.zeros((128, MCAP * U), np.float32)
        mymeta = []
        for j in range(U):
            gidx = ranks[j] * N_CORES + c
            if gidx >= len(units):
                continue
            L, its = units[gidx]
            assert L <= widths[j]
            for i, (b, k, xoff, xlen, h, p0) in enumerate(its):
                yv = (128 * k + np.arange(h)).astype(np.float64)
                z = yv / ilf[b]
                Aq = 2.0 * KC * z / olf[b]          # x coeff
                Cq = -KC / (olf[b] ** 2)            # x^2 coeff
                Ap = Aq + 2.0 * xoff * Cq
                biasp = -KC * z * z + Aq * xoff + Cq * xoff * xoff
                AC[0, 128 * j + p0:128 * j + p0 + h] = Ap
                AC[1, 128 * j + p0:128 * j + p0 + h] = 4096.0 * Cq
                AC[2, 128 * j + p0:128 * j + p0 + h] = biasp
                IND[p0:p0 + h, MCAP * j + i] = 1.0
                Ar[p0:p0 + h, offs[j]:offs[j] + xlen] = \
                    att_ws[b, xoff:xoff + xlen, 128 * k:128 * k + h].T
                mymeta.append((j, i, b, p0, h))
        in_maps.append({
            "A": Ar.astype(ml_dtypes.bfloat16),
            "RC": np.concatenate([RAMP, AC], axis=1).astype(np.float16),
            "IND": IND.astype(ml_dtypes.bfloat16),
        })
        meta.append(mymeta)

    key = (tuple(widths), plan)
    if key not in _cache:
        _cache[key] = _build_program(list(widths), plan, 8)
    nc = _cache[key]
    kw = {}
    if _trace:
        kw = dict(trace=True, tmpdir=_tracedir)
    res = run_bass_kernel_spmd(nc, in_maps, list(range(N_CORES)), **kw)

    sum1 = np.zeros(B, np.float64)
    sum2 = np.zeros(B, np.float64)
    for c in range(N_CORES):
        r1 = np.asarray(res.results[c]["R1"], np.float64)
        st = np.asarray(res.results[c]["STR"], np.float64)
        for (j, i, b, p0, h) in meta[c]:
            sum1[b] += r1[p0:p0 + h, j].sum()
            g, r = j // 32, j % 32
            sr, sc = 32 * (r % 4), 64 * (r // 4)
            sum2[b] += st[sr + i, 512 * g + sc:512 * g + sc + 64].sum()

    l1 = (-sum1 / olf).astype(np.float32)
    l2 = (sum2 / olf).astype(np.float32)
    return (l1, l2)


# revision 5
# speedup vs baseline: 1.1132x; 1.0350x over previous
"""GuidedAttentionLoss on 8 TRN2 cores — transposed-layout pipeline using
all five engines.

Layout: partition dim = input tokens y, free dim = output frames x
(att_ws transposed per sample). Work is packed into 128-partition "units"
(stacks of (sample, y-block, x-piece) items, first-fit-decreasing on
height), striped round-robin across the 8 cores with shared program
geometry (per-stripe widths = max over cores, emission ordered small ->
big -> small for fast pipeline fill/drain).

Per unit (L = padded x-width, L <= 1536):
  PE:   arg[p,x] = A_p*x + C_p*(x^2/4096) + bias_p  (K=3 fp16 matmul
        against a shared [x, x^2/4096, 1] ramp; the per-partition coeffs
        fold the full guided-attention quadratic -3.125*(y/il - x/ol)^2
        including x-piece offsets; bias rides as a third contraction row)
  Act:  e = Exp(arg)                                 (PSUM -> SBUF bf16)
  DVE:  t = (e - 1)*a   scalar_tensor_tensor, accum_out -> r1[:, unit]
  Pool: q = t*t         (gpsimd tensor_tensor, bf16)
  PE:   strip[slot] += ind.T @ q   (64-wide accumulating indicator
        matmuls -> per-item partial column sums in a PSUM strip bank)
Strips are staged to SBUF (DVE copy) and DMA'd out in 128-col blocks as
they complete; the host does the final tiny per-item gather in f64.
The coeff/ramp aux load is split so PE can start ~4.3us in; the Act
exp-table load is warmed by a dummy activation during the DMA fill.

Input att_ws is cast to bf16 on the host (rel tol 2e-2; measured total
error ~6e-4). HW exec time (CoreSim cost model, same inputs): 29499 ns
vs 81416 ns for the previous width-class row-layout kernel (2.76x).
"""
import numpy as np
import ml_dtypes

N_CORES = 8
SIGMA = 0.4
KC = 1.0 / (2 * SIGMA * SIGMA)  # 3.125
LMAX = 1536        # x-piece cap (arg psum tile: 3 banks)
MCAP = 8           # items per unit cap (ind columns / strip slot rows)
XMAX = 2048
LQUANT = 32

_cache = {}


# --------------------------------------------------------------- packing ---
def _pack(il, ol):
    """Build global units. Returns list of units; each unit:
    (L, items) with items = list of (b, k, xoff, xlen, h, p0)."""
    items = []
    B = len(il)
    for b in range(B):
        nb = (int(il[b]) + 127) // 128
        for k in range(nb):
            h = min(int(il[b]) - 128 * k, 128)
            xoff = 0
            while xoff < int(ol[b]):
                xlen = min(LMAX, int(ol[b]) - xoff)
                items.append((b, k, xoff, xlen, h))
                xoff += xlen
    # first-fit-decreasing on partition height, items in xlen-desc order so
    # a unit's L (max xlen = first item's xlen) never grows when stacking
    items.sort(key=lambda t: -t[3])
    units = []       # list of item lists
    room = []        # partition rows left per unit
    for it in items:
        h = it[4]
        for i, r in enumerate(room):
            if r >= h and len(units[i]) < MCAP:
                units[i].append(it)
                room[i] -= h
                break
        else:
            units.append([it])
            room.append(128 - h)
    out = []
    for u in units:
        L = -(-max(it[3] for it in u) // LQUANT) * LQUANT
        p0 = 0
        its = []
        for (b, k, xoff, xlen, h) in u:
            its.append((b, k, xoff, xlen, h, p0))
            p0 += h
        out.append((L, its))
    # sort units by L desc for striping
    out.sort(key=lambda t: -t[0])
    return out


def _stripe(units):
    """Assign global unit j -> (core j%8, stripe rank j//8), then permute
    stripe emission order: a couple of small stripes first (fast pipeline
    fill), the big ones in the middle, small again at the end (fast drain).
    Returns (U, widths_in_emission_order, ranks) with ranks[slot] = stripe
    rank (desc-width index) emitted at that slot."""
    U = (len(units) + N_CORES - 1) // N_CORES
    desc = [units[s * N_CORES][0] for s in range(U)]  # desc widths
    order = []
    if U > 3:
        order = [U - 1, U - 3] + list(range(U - 3)) + [U - 2]
    else:
        order = list(range(U))
    widths = [desc[r] for r in order]
    return U, widths, tuple(order)


def _engine_plan(widths):
    """Per stripe: 'v' (DVE) or 'g' (Pool) for the q = t*t pass, balancing
    measured busy rates (Pool TT costs 0.833 ns/col in the cost model; DVE
    TT 0.52 but DVE already runs every stt at 1.04)."""
    dve = sum(w * 1.0417 for w in widths) + 1300.0  # stt passes + copies
    pool = 0.0
    plan = []
    for w in widths:
        if dve + w * 0.5208 + 400.0 < pool + w * 0.8333 + 95.0:
            plan.append("v")
            dve += w * 0.5208
        else:
            plan.append("g")
            pool += w * 0.8333 + 95.0
    return tuple(plan)


# --------------------------------------------------------------- program ---
def _build_program(widths, plan, nchunks_dma):
    import concourse.bacc as bacc
    import concourse.mybir as mybir
    import concourse.tile as tile

    F32 = mybir.dt.float32
    BF16 = mybir.dt.bfloat16
    FP16 = mybir.dt.float16
    Exp = mybir.ActivationFunctionType.Exp
    Copy = mybir.ActivationFunctionType.Copy
    sub = mybir.AluOpType.subtract
    mult = mybir.AluOpType.mult

    U = len(widths)
    F = sum(widths)
    G = (U + 31) // 32  # strip generations

    # group stripes into DMA chunks: small first chunks so compute starts
    # early, growing to ~6K cols (12KB/partition, under the 64KB desc cap)
    chunks = []  # (u_start, u_end, col_off, width)
    u0, c0 = 0, 0
    targets = [512, 1536, 3072]  # then 6144
    for u in range(U + 1):
        w = sum(widths[u0:u])
        tgt = targets[len(chunks)] if len(chunks) < len(targets) else 6144
        if u == U or (w >= tgt and u > u0):
            if u > u0:
                chunks.append((u0, u, c0, w))
                c0 += w
                u0 = u

    nc = bacc.Bacc("TRN2", target_bir_lowering=False, debug=False,
                   num_devices=1)
    A = nc.declare_dram_parameter("A", [128, F], BF16, isOutput=False)
    # RC = [x-ramp rows (x, x^2/4096, 1) | per-unit (A,C,bias) coeff
    # columns], one fp16 aux load; bias rides the matmul as a K=3 row
    RCA = nc.declare_dram_parameter("RCA", [3, 128 * U + 512], FP16,
                                    isOutput=False)
    RCB = nc.declare_dram_parameter("RCB", [3, XMAX - 512], FP16,
                                    isOutput=False)
    IND = nc.declare_dram_parameter("IND", [128, MCAP * U], BF16,
                                    isOutput=False)
    R1 = nc.declare_dram_parameter("R1", [128, U], F32, isOutput=True)
    STR = nc.declare_dram_parameter("STR", [128, 512 * G], F32, isOutput=True)

    with tile.TileContext(nc) as tc:
        with tc.tile_pool(name="aux", bufs=1) as aux, \
             tc.tile_pool(name="pa", bufs=1) as pa, \
             tc.tile_pool(name="pe_", bufs=4) as pev, \
             tc.tile_pool(name="pt", bufs=4) as pt, \
             tc.tile_pool(name="pq", bufs=4) as pq, \
             tc.tile_pool(name="psarg", bufs=2, space="PSUM") as psarg, \
             tc.tile_pool(name="psstr", bufs=2, space="PSUM") as psstr:
            # issue order matters: rc (PE needs it first), then the first
            # small a-chunk (stt), then bias/ind, then remaining chunks
            warm = aux.tile([128, 1], F32)
            nc.vector.memset(warm[:], 0.0)
            warm2 = aux.tile([128, 1], F32)
            nc.scalar.activation(warm2[:], warm[:], Exp, scale=1.0)
            rca = aux.tile([3, 128 * U + 512], FP16)
            nc.sync.dma_start(rca[:], RCA[:])
            rcb = aux.tile([3, XMAX - 512], FP16)
            nc.sync.dma_start(rcb[:], RCB[:])

            def ramp_sl(c, cw):
                if c + cw <= 512:
                    return rca[:, 128 * U + c:128 * U + c + cw]
                return rcb[:, c - 512:c - 512 + cw]
            ind = aux.tile([128, MCAP * U], BF16)
            r1 = aux.tile([128, U], F32)

            # chunked input loads (separate tiles so compute can start early)
            atiles = []
            for ci, (us, ue, coff, w) in enumerate(chunks):
                t_ = pa.tile([128, w], BF16, tag=f"a{us}")
                nc.sync.dma_start(t_[:], A[:, coff:coff + w])
                atiles.append((us, ue, coff, t_))
                if ci == 0:
                    nc.sync.dma_start(ind[:], IND[:])

            def a_slice(u, off_u, L):
                for (us, ue, coff, t_) in atiles:
                    if us <= u < ue:
                        return t_[:, off_u - coff:off_u - coff + L]
                raise AssertionError

            strip = None
            stage = None
            off = 0
            # xoff per stripe is geometry: ramp slice start. xoff is the
            # SAME for all items in a unit? No - items have different xoff.
            # arg uses absolute x: ramp slice must match each item's x range.
            # => all items in a unit must share the same x-window [xw, xw+L).
            # Handled in packing: xoff passed via widths-aligned plan...
            # (see _pack2: units carry xw; here xw comes in via plan tuple)
            # pair consecutive <=768-wide units: both args share one psum
            # tile (B at column 768) and a single Act Exp covers both
            pair_first = {}
            pair_second = {}
            # (pairing disabled: measured slower than per-unit Act)

            et_pair = None
            for u in range(U):
                L = widths[u]
                g, r = u // 32, u % 32
                sr, sc = 32 * (r % 4), 64 * (r // 4)
                if r == 0:
                    strip = psstr.tile([128, 512], F32, tag="strip")
                    nc.vector.memset(strip[:], 0.0)
                if u in pair_second:
                    et = et_pair[:, 768:768 + L]
                elif u in pair_first:
                    ub = pair_first[u]
                    lb = widths[ub]
                    arg = psarg.tile([128, LMAX], F32, tag="arg")
                    # A's chunks extended to the full 768 cols: the gap
                    # [L_A, 768) computes real arg values (always <= 0, so
                    # exp <= 1) that nothing reads — keeps psum initialized
                    for uu, base, ext in ((u, 0, 768), (ub, 768, lb)):
                        for c in range(0, ext, 512):
                            cw = min(512, ext - c)
                            bb = base + c
                            cw = min(cw, 512 - (bb % 512)) if bb % 512 \
                                else cw
                            nc.tensor.matmul(
                                arg[:, bb:bb + cw],
                                rc[:, XMAX + 128 * uu:XMAX + 128 * uu + 128],
                                ramp[:, c:c + cw],
                                start=True, stop=True)
                            if cw < min(512, ext - c):
                                c2 = c + cw
                                cw2 = min(512, ext - c) - cw
                                nc.tensor.matmul(
                                    arg[:, base + c2:base + c2 + cw2],
                                    rc[:, XMAX + 128 * uu:
                                        XMAX + 128 * uu + 128],
                                    ramp[:, c2:c2 + cw2],
                                    start=True, stop=True)
                    et_pair = pev.tile([128, 768 + lb], BF16, tag="e")
                    nc.scalar.activation(et_pair[:], arg[:, :768 + lb],
                                         Exp, scale=1.0)
                    et = et_pair[:, :L]
                else:
                    arg = psarg.tile([128, LMAX], F32, tag="arg")
                    for c in range(0, L, 512):
                        cw = min(512, L - c)
                        nc.tensor.matmul(
                            arg[:, c:c + cw],
                            rca[:, 128 * u:128 * u + 128],
                            ramp_sl(c, cw),
                            start=True, stop=True)
                    et_full = pev.tile([128, L], BF16, tag="e")
                    nc.scalar.activation(et_full[:], arg[:, :L], Exp,
                                         scale=1.0)
                    et = et_full[:]
                tt = pt.tile([128, L], BF16, tag="t")
                nc.vector.scalar_tensor_tensor(
                    tt[:], et[:], 1.0, a_slice(u, off, L), sub, mult,
                    accum_out=r1[:, u:u + 1])
                qt = pq.tile([128, L], BF16, tag="q")
                qeng = nc.vector if plan[u] == "v" else nc.gpsimd
                qeng.tensor_tensor(qt[:], tt[:], tt[:], mult)
                nch = (L + 63) // 64
                for ci, c in enumerate(range(0, L, 64)):
                    cw = min(64, L - c)
                    nc.tensor.matmul(
                        strip[sr:sr + MCAP, sc:sc + cw],
                        ind[:, MCAP * u:MCAP * u + MCAP],
                        qt[:, c:c + cw],
                        start=(ci == 0), stop=(ci == nch - 1),
                        tile_position=(0, sr))
                last = u == U - 1
                if (r % 8 == 7 and not last) or (last and r % 8 != 7):
                    # flush completed 128-col block(s) of this generation
                    lo = 128 * (r // 8)
                    hi = lo + 128 if not last else 64 * (r // 4) + 64
                    sh = pev.tile([128, 512], F32, tag="stg")
                    nc.vector.tensor_copy(sh[:, lo:hi], strip[:, lo:hi])
                    nc.sync.dma_start(STR[:, 512 * g + lo:512 * g + hi],
                                      sh[:, lo:hi])
                elif last:
                    lo = 128 * (r // 8)
                    hi = 64 * (r // 4) + 64
                    sh = pev.tile([128, 512], F32, tag="stg")
                    nc.vector.tensor_copy(sh[:, lo:hi], strip[:, lo:hi])
                    nc.sync.dma_start(STR[:, 512 * g + lo:512 * g + hi],
                                      sh[:, lo:hi])
                if u == (3 * U) // 4:
                    usplit = u + 1
                    nc.sync.dma_start(R1[:, :usplit], r1[:, :usplit])
                off += L

            nc.scalar.dma_start(R1[:, usplit:], r1[:, usplit:])
    nc.compile()
    return nc


# ---------------------------------------------------------------- kernel ---
def kernel(att_ws, ilens, olens, _trace=False, _tracedir=None):
    from concourse.bass_utils import run_bass_kernel_spmd

    att_ws = np.asarray(att_ws)
    il = np.asarray(ilens).astype(np.int64)
    ol = np.asarray(olens).astype(np.int64)
    B, T_out, T_in = att_ws.shape

    units = _pack(il, ol)
    U, widths, ranks = _stripe(units)
    plan = _engine_plan(widths)
    G = (U + 31) // 32
    F = sum(widths)

    # x-window consistency: items within a global unit may have different
    # xoff. The program indexes ramp[:, 0:L] (x starting at 0). For items
    # with xoff > 0 we instead fold the shift into per-partition constants:
    # arg(x') = A_p*(x'+xoff) + C_p*(x'+xoff)^2/..  -- quadratic in x' with
    # modified A'_p, C'_p (C unchanged), bias absorbs constant:
    #   A'_p = A_p + 2*xoff*Cq_p ; bias' += A_p*xoff + Cq_p*xoff^2
    # where Cq_p = -KC/ol^2 (true x^2 coeff; C_p = 4096*Cq_p).
    ilf = il.astype(np.float64)
    olf = ol.astype(np.float64)

    offs = np.concatenate([[0], np.cumsum(widths)]).astype(np.int64)
    in_maps = []
    meta = []  # per core: list of (j, item list)
    x = np.arange(XMAX, dtype=np.float64)
    RAMP = np.stack([x, x * x / 4096.0, np.ones(XMAX)])

    for c in range(N_CORES):
        Ar = np.zeros((128, F), np.float32)
        AC = np.zeros((3, 128 * U), np.float32)
        IND = np.zeros((128, MCAP * U), np.float32)
        mymeta = []
        for j in range(U):
            gidx = ranks[j] * N_CORES + c
            if gidx >= len(units):
                continue
            L, its = units[gidx]
            assert L <= widths[j]
            for i, (b, k, xoff, xlen, h, p0) in enumerate(its):
                yv = (128 * k + np.arange(h)).astype(np.float64)
                z = yv / ilf[b]
                Aq = 2.0 * KC * z / olf[b]          # x coeff
                Cq = -KC / (olf[b] ** 2)            # x^2 coeff
                Ap = Aq + 2.0 * xoff * Cq
                biasp = -KC * z * z + Aq * xoff + Cq * xoff * xoff
                AC[0, 128 * j + p0:128 * j + p0 + h] = Ap
                AC[1, 128 * j + p0:128 * j + p0 + h] = 4096.0 * Cq
                AC[2, 128 * j + p0:128 * j + p0 + h] = biasp
                IND[p0:p0 + h, MCAP * j + i] = 1.0
                Ar[p0:p0 + h, offs[j]:offs[j] + xlen] = \
                    att_ws[b, xoff:xoff + xlen, 128 * k:128 * k + h].T
                mymeta.append((j, i, b, p0, h))
        in_maps.append({
            "A": Ar.astype(ml_dtypes.bfloat16),
            "RCA": np.concatenate([AC, RAMP[:, :512]],
                                  axis=1).astype(np.float16),
            "RCB": RAMP[:, 512:].astype(np.float16),
            "IND": IND.astype(ml_dtypes.bfloat16),
        })
        meta.append(mymeta)

    key = (tuple(widths), plan)
    if key not in _cache:
        _cache[key] = _build_program(list(widths), plan, 8)
    nc = _cache[key]
    kw = {}
    if _trace:
        kw = dict(trace=True, tmpdir=_tracedir)
    res = run_bass_kernel_spmd(nc, in_maps, list(range(N_CORES)), **kw)

    sum1 = np.zeros(B, np.float64)
    sum2 = np.zeros(B, np.float64)
    for c in range(N_CORES):
        r1 = np.asarray(res.results[c]["R1"], np.float64)
        st = np.asarray(res.results[c]["STR"], np.float64)
        for (j, i, b, p0, h) in meta[c]:
            sum1[b] += r1[p0:p0 + h, j].sum()
            g, r = j // 32, j % 32
            sr, sc = 32 * (r % 4), 64 * (r // 4)
            sum2[b] += st[sr + i, 512 * g + sc:512 * g + sc + 64].sum()

    l1 = (-sum1 / olf).astype(np.float32)
    l2 = (sum2 / olf).astype(np.float32)
    return (l1, l2)


# revision 6
# speedup vs baseline: 1.1615x; 1.0433x over previous
"""GuidedAttentionLoss on 8 TRN2 cores — transposed-layout pipeline using
all five engines.

Layout: partition dim = input tokens y, free dim = output frames x
(att_ws transposed per sample). Work is packed into 128-partition "units"
(stacks of (sample, y-block, x-piece) items, first-fit-decreasing on
height), striped round-robin across the 8 cores with shared program
geometry (per-stripe widths = max over cores, emission ordered small ->
big -> small for fast pipeline fill/drain).

Per unit (L = padded x-width, L <= 1536):
  PE:   arg[p,x] = A_p*x + C_p*(x^2/4096) + bias_p  (K=3 fp16 matmul
        against a shared [x, x^2/4096, 1] ramp; the per-partition coeffs
        fold the full guided-attention quadratic -3.125*(y/il - x/ol)^2
        including x-piece offsets; bias rides as a third contraction row)
  Act:  e = Exp(arg)                                 (PSUM -> SBUF bf16)
  DVE:  t = (e - 1)*a   scalar_tensor_tensor, accum_out -> r1[:, unit]
  Pool: q = t*t         (gpsimd tensor_tensor, bf16)
  PE:   strip[slot] += ind.T @ q   (64-wide accumulating indicator
        matmuls -> per-item partial column sums in a PSUM strip bank)
Strips are staged to SBUF (DVE copy) and DMA'd out in 128-col blocks as
they complete; the host does the final tiny per-item gather in f64.
The coeff/ramp aux load is split so PE can start ~4.3us in; the Act
exp-table load is warmed by a dummy activation during the DMA fill, and
the PE clock is pre-ramped to full p-state by dummy matmuls into the
strip bank (overwritten by its memset) so the first real args run at
2.4 GHz instead of mid-clock.

Input att_ws is cast to bf16 on the host (rel tol 2e-2; measured total
error ~6e-4). HW exec time (CoreSim cost model, same inputs): 28502 ns
vs 81416 ns for the previous width-class row-layout kernel (2.86x).
"""
import numpy as np
import ml_dtypes

N_CORES = 8
SIGMA = 0.4
KC = 1.0 / (2 * SIGMA * SIGMA)  # 3.125
LMAX = 1536        # x-piece cap (arg psum tile: 3 banks)
MCAP = 8           # items per unit cap (ind columns / strip slot rows)
XMAX = 2048
LQUANT = 32

_cache = {}


# --------------------------------------------------------------- packing ---
def _pack(il, ol):
    """Build global units. Returns list of units; each unit:
    (L, items) with items = list of (b, k, xoff, xlen, h, p0)."""
    items = []
    B = len(il)
    for b in range(B):
        nb = (int(il[b]) + 127) // 128
        for k in range(nb):
            h = min(int(il[b]) - 128 * k, 128)
            xoff = 0
            while xoff < int(ol[b]):
                xlen = min(LMAX, int(ol[b]) - xoff)
                items.append((b, k, xoff, xlen, h))
                xoff += xlen
    # first-fit-decreasing on partition height, items in xlen-desc order so
    # a unit's L (max xlen = first item's xlen) never grows when stacking
    items.sort(key=lambda t: -t[3])
    units = []       # list of item lists
    room = []        # partition rows left per unit
    for it in items:
        h = it[4]
        for i, r in enumerate(room):
            if r >= h and len(units[i]) < MCAP:
                units[i].append(it)
                room[i] -= h
                break
        else:
            units.append([it])
            room.append(128 - h)
    out = []
    for u in units:
        L = -(-max(it[3] for it in u) // LQUANT) * LQUANT
        p0 = 0
        its = []
        for (b, k, xoff, xlen, h) in u:
            its.append((b, k, xoff, xlen, h, p0))
            p0 += h
        out.append((L, its))
    # sort units by L desc for striping
    out.sort(key=lambda t: -t[0])
    return out


def _stripe(units):
    """Assign global unit j -> (core j%8, stripe rank j//8), then permute
    stripe emission order: a couple of small stripes first (fast pipeline
    fill), the big ones in the middle, small again at the end (fast drain).
    Returns (U, widths_in_emission_order, ranks) with ranks[slot] = stripe
    rank (desc-width index) emitted at that slot."""
    U = (len(units) + N_CORES - 1) // N_CORES
    desc = [units[s * N_CORES][0] for s in range(U)]  # desc widths
    order = []
    if U > 3:
        order = [U - 1, U - 3] + list(range(U - 3)) + [U - 2]
    else:
        order = list(range(U))
    widths = [desc[r] for r in order]
    return U, widths, tuple(order)


def _engine_plan(widths):
    """Per stripe: 'v' (DVE) or 'g' (Pool) for the q = t*t pass, balancing
    measured busy rates (Pool TT costs 0.833 ns/col in the cost model; DVE
    TT 0.52 but DVE already runs every stt at 1.04)."""
    dve = sum(w * 1.0417 for w in widths) + 1300.0  # stt passes + copies
    pool = 0.0
    plan = []
    for w in widths:
        if dve + w * 0.5208 + 400.0 < pool + w * 0.8333 + 95.0:
            plan.append("v")
            dve += w * 0.5208
        else:
            plan.append("g")
            pool += w * 0.8333 + 95.0
    return tuple(plan)


# --------------------------------------------------------------- program ---
def _build_program(widths, plan, nchunks_dma):
    import concourse.bacc as bacc
    import concourse.mybir as mybir
    import concourse.tile as tile

    F32 = mybir.dt.float32
    BF16 = mybir.dt.bfloat16
    FP16 = mybir.dt.float16
    Exp = mybir.ActivationFunctionType.Exp
    Copy = mybir.ActivationFunctionType.Copy
    sub = mybir.AluOpType.subtract
    mult = mybir.AluOpType.mult

    U = len(widths)
    F = sum(widths)
    G = (U + 31) // 32  # strip generations

    # group stripes into DMA chunks: small first chunks so compute starts
    # early, growing to ~6K cols (12KB/partition, under the 64KB desc cap)
    chunks = []  # (u_start, u_end, col_off, width)
    u0, c0 = 0, 0
    targets = [512, 1536, 3072]  # then 6144
    for u in range(U + 1):
        w = sum(widths[u0:u])
        tgt = targets[len(chunks)] if len(chunks) < len(targets) else 6144
        if u == U or (w >= tgt and u > u0):
            if u > u0:
                chunks.append((u0, u, c0, w))
                c0 += w
                u0 = u

    nc = bacc.Bacc("TRN2", target_bir_lowering=False, debug=False,
                   num_devices=1)
    A = nc.declare_dram_parameter("A", [128, F], BF16, isOutput=False)
    # RC = [x-ramp rows (x, x^2/4096, 1) | per-unit (A,C,bias) coeff
    # columns], one fp16 aux load; bias rides the matmul as a K=3 row
    RCA = nc.declare_dram_parameter("RCA", [3, 128 * U + 512], FP16,
                                    isOutput=False)
    RCB = nc.declare_dram_parameter("RCB", [3, XMAX - 512], FP16,
                                    isOutput=False)
    IND = nc.declare_dram_parameter("IND", [128, MCAP * U], BF16,
                                    isOutput=False)
    R1 = nc.declare_dram_parameter("R1", [128, U], F32, isOutput=True)
    STR = nc.declare_dram_parameter("STR", [128, 512 * G], F32, isOutput=True)

    with tile.TileContext(nc) as tc:
        with tc.tile_pool(name="aux", bufs=1) as aux, \
             tc.tile_pool(name="pa", bufs=1) as pa, \
             tc.tile_pool(name="pe_", bufs=4) as pev, \
             tc.tile_pool(name="pt", bufs=4) as pt, \
             tc.tile_pool(name="pq", bufs=4) as pq, \
             tc.tile_pool(name="psarg", bufs=2, space="PSUM") as psarg, \
             tc.tile_pool(name="psstr", bufs=2, space="PSUM") as psstr:
            # issue order matters: rc (PE needs it first), then the first
            # small a-chunk (stt), then bias/ind, then remaining chunks
            warm = aux.tile([128, 1], F32)
            nc.vector.memset(warm[:], 0.0)
            warm2 = aux.tile([128, 1], F32)
            nc.scalar.activation(warm2[:], warm[:], Exp, scale=1.0)
            # PE p-state warm-up: keep the tensor engine busy from t~1.5us
            # so the first real args run at full clock. Dummies write the
            # gen-0 strip bank, which is memset afterwards anyway.
            wpe = aux.tile([2, 512], FP16)
            nc.vector.memset(wpe[:], 0.0)
            strip0 = psstr.tile([128, 512], F32, tag="strip")
            for _ in range(5):
                nc.tensor.matmul(strip0[:, :512], wpe[:, :128],
                                 wpe[:, :512], start=True, stop=True)
            rca = aux.tile([3, 128 * U + 512], FP16)
            nc.sync.dma_start(rca[:], RCA[:])
            rcb = aux.tile([3, XMAX - 512], FP16)
            nc.sync.dma_start(rcb[:], RCB[:])

            def ramp_sl(c, cw):
                if c + cw <= 512:
                    return rca[:, 128 * U + c:128 * U + c + cw]
                return rcb[:, c - 512:c - 512 + cw]
            ind = aux.tile([128, MCAP * U], BF16)
            r1 = aux.tile([128, U], F32)

            # chunked input loads (separate tiles so compute can start early)
            atiles = []
            for ci, (us, ue, coff, w) in enumerate(chunks):
                t_ = pa.tile([128, w], BF16, tag=f"a{us}")
                nc.sync.dma_start(t_[:], A[:, coff:coff + w])
                atiles.append((us, ue, coff, t_))
                if ci == 0:
                    nc.sync.dma_start(ind[:], IND[:])

            def a_slice(u, off_u, L):
                for (us, ue, coff, t_) in atiles:
                    if us <= u < ue:
                        return t_[:, off_u - coff:off_u - coff + L]
                raise AssertionError

            strip = None
            stage = None
            off = 0
            # xoff per stripe is geometry: ramp slice start. xoff is the
            # SAME for all items in a unit? No - items have different xoff.
            # arg uses absolute x: ramp slice must match each item's x range.
            # => all items in a unit must share the same x-window [xw, xw+L).
            # Handled in packing: xoff passed via widths-aligned plan...
            # (see _pack2: units carry xw; here xw comes in via plan tuple)
            # pair consecutive <=768-wide units: both args share one psum
            # tile (B at column 768) and a single Act Exp covers both
            pair_first = {}
            pair_second = {}
            # (pairing disabled: measured slower than per-unit Act)

            et_pair = None
            for u in range(U):
                L = widths[u]
                g, r = u // 32, u % 32
                sr, sc = 32 * (r % 4), 64 * (r // 4)
                if r == 0:
                    strip = strip0 if g == 0 else \
                        psstr.tile([128, 512], F32, tag="strip")
                    nc.vector.memset(strip[:], 0.0)
                if u in pair_second:
                    et = et_pair[:, 768:768 + L]
                elif u in pair_first:
                    ub = pair_first[u]
                    lb = widths[ub]
                    arg = psarg.tile([128, LMAX], F32, tag="arg")
                    # A's chunks extended to the full 768 cols: the gap
                    # [L_A, 768) computes real arg values (always <= 0, so
                    # exp <= 1) that nothing reads — keeps psum initialized
                    for uu, base, ext in ((u, 0, 768), (ub, 768, lb)):
                        for c in range(0, ext, 512):
                            cw = min(512, ext - c)
                            bb = base + c
                            cw = min(cw, 512 - (bb % 512)) if bb % 512 \
                                else cw
                            nc.tensor.matmul(
                                arg[:, bb:bb + cw],
                                rc[:, XMAX + 128 * uu:XMAX + 128 * uu + 128],
                                ramp[:, c:c + cw],
                                start=True, stop=True)
                            if cw < min(512, ext - c):
                                c2 = c + cw
                                cw2 = min(512, ext - c) - cw
                                nc.tensor.matmul(
                                    arg[:, base + c2:base + c2 + cw2],
                                    rc[:, XMAX + 128 * uu:
                                        XMAX + 128 * uu + 128],
                                    ramp[:, c2:c2 + cw2],
                                    start=True, stop=True)
                    et_pair = pev.tile([128, 768 + lb], BF16, tag="e")
                    nc.scalar.activation(et_pair[:], arg[:, :768 + lb],
                                         Exp, scale=1.0)
                    et = et_pair[:, :L]
                else:
                    arg = psarg.tile([128, LMAX], F32, tag="arg")
                    for c in range(0, L, 512):
                        cw = min(512, L - c)
                        nc.tensor.matmul(
                            arg[:, c:c + cw],
                            rca[:, 128 * u:128 * u + 128],
                            ramp_sl(c, cw),
                            start=True, stop=True)
                    et_full = pev.tile([128, L], BF16, tag="e")
                    nc.scalar.activation(et_full[:], arg[:, :L], Exp,
                                         scale=1.0)
                    et = et_full[:]
                tt = pt.tile([128, L], BF16, tag="t")
                nc.vector.scalar_tensor_tensor(
                    tt[:], et[:], 1.0, a_slice(u, off, L), sub, mult,
                    accum_out=r1[:, u:u + 1])
                qt = pq.tile([128, L], BF16, tag="q")
                qeng = nc.vector if plan[u] == "v" else nc.gpsimd
                qeng.tensor_tensor(qt[:], tt[:], tt[:], mult)
                nch = (L + 63) // 64
                for ci, c in enumerate(range(0, L, 64)):
                    cw = min(64, L - c)
                    nc.tensor.matmul(
                        strip[sr:sr + MCAP, sc:sc + cw],
                        ind[:, MCAP * u:MCAP * u + MCAP],
                        qt[:, c:c + cw],
                        start=(ci == 0), stop=(ci == nch - 1),
                        tile_position=(0, sr))
                last = u == U - 1
                if (r % 8 == 7 and not last) or (last and r % 8 != 7):
                    # flush completed 128-col block(s) of this generation
                    lo = 128 * (r // 8)
                    hi = lo + 128 if not last else 64 * (r // 4) + 64
                    sh = pev.tile([128, 512], F32, tag="stg")
                    nc.vector.tensor_copy(sh[:, lo:hi], strip[:, lo:hi])
                    nc.sync.dma_start(STR[:, 512 * g + lo:512 * g + hi],
                                      sh[:, lo:hi])
                elif last:
                    lo = 128 * (r // 8)
                    hi = 64 * (r // 4) + 64
                    sh = pev.tile([128, 512], F32, tag="stg")
                    nc.vector.tensor_copy(sh[:, lo:hi], strip[:, lo:hi])
                    nc.sync.dma_start(STR[:, 512 * g + lo:512 * g + hi],
                                      sh[:, lo:hi])
                if u == (3 * U) // 4:
                    usplit = u + 1
                    nc.sync.dma_start(R1[:, :usplit], r1[:, :usplit])
                off += L

            nc.scalar.dma_start(R1[:, usplit:], r1[:, usplit:])
    nc.compile()
    return nc


# ---------------------------------------------------------------- kernel ---
def kernel(att_ws, ilens, olens, _trace=False, _tracedir=None):
    from concourse.bass_utils import run_bass_kernel_spmd

    att_ws = np.asarray(att_ws)
    il = np.asarray(ilens).astype(np.int64)
    ol = np.asarray(olens).astype(np.int64)
    B, T_out, T_in = att_ws.shape

    units = _pack(il, ol)
    U, widths, ranks = _stripe(units)
    plan = _engine_plan(widths)
    G = (U + 31) // 32
    F = sum(widths)

    # x-window consistency: items within a global unit may have different
    # xoff. The program indexes ramp[:, 0:L] (x starting at 0). For items
    # with xoff > 0 we instead fold the shift into per-partition constants:
    # arg(x') = A_p*(x'+xoff) + C_p*(x'+xoff)^2/..  -- quadratic in x' with
    # modified A'_p, C'_p (C unchanged), bias absorbs constant:
    #   A'_p = A_p + 2*xoff*Cq_p ; bias' += A_p*xoff + Cq_p*xoff^2
    # where Cq_p = -KC/ol^2 (true x^2 coeff; C_p = 4096*Cq_p).
    ilf = il.astype(np.float64)
    olf = ol.astype(np.float64)

    offs = np.concatenate([[0], np.cumsum(widths)]).astype(np.int64)
    in_maps = []
    meta = []  # per core: list of (j, item list)
    x = np.arange(XMAX, dtype=np.float64)
    RAMP = np.stack([x, x * x / 4096.0, np.ones(XMAX)])

    for c in range(N_CORES):
        Ar = np.zeros((128, F), np.float32)
        AC = np.zeros((3, 128 * U), np.float32)
        IND = np.zeros((128, MCAP * U), np.float32)
        mymeta = []
        for j in range(U):
            gidx = ranks[j] * N_CORES + c
            if gidx >= len(units):
                continue
            L, its = units[gidx]
            assert L <= widths[j]
            for i, (b, k, xoff, xlen, h, p0) in enumerate(its):
                yv = (128 * k + np.arange(h)).astype(np.float64)
                z = yv / ilf[b]
                Aq = 2.0 * KC * z / olf[b]          # x coeff
                Cq = -KC / (olf[b] ** 2)            # x^2 coeff
                Ap = Aq + 2.0 * xoff * Cq
                biasp = -KC * z * z + Aq * xoff + Cq * xoff * xoff
                AC[0, 128 * j + p0:128 * j + p0 + h] = Ap
                AC[1, 128 * j + p0:128 * j + p0 + h] = 4096.0 * Cq
                AC[2, 128 * j + p0:128 * j + p0 + h] = biasp
                IND[p0:p0 + h, MCAP * j + i] = 1.0
                Ar[p0:p0 + h, offs[j]:offs[j] + xlen] = \
                    att_ws[b, xoff:xoff + xlen, 128 * k:128 * k + h].T
                mymeta.append((j, i, b, p0, h))
        in_maps.append({
            "A": Ar.astype(ml_dtypes.bfloat16),
            "RCA": np.concatenate([AC, RAMP[:, :512]],
                                  axis=1).astype(np.float16),
            "RCB": RAMP[:, 512:].astype(np.float16),
            "IND": IND.astype(ml_dtypes.bfloat16),
        })
        meta.append(mymeta)

    key = (tuple(widths), plan)
    if key not in _cache:
        _cache[key] = _build_program(list(widths), plan, 8)
    nc = _cache[key]
    kw = {}
    if _trace:
        kw = dict(trace=True, tmpdir=_tracedir)
    res = run_bass_kernel_spmd(nc, in_maps, list(range(N_CORES)), **kw)

    sum1 = np.zeros(B, np.float64)
    sum2 = np.zeros(B, np.float64)
    for c in range(N_CORES):
        r1 = np.asarray(res.results[c]["R1"], np.float64)
        st = np.asarray(res.results[c]["STR"], np.float64)
        for (j, i, b, p0, h) in meta[c]:
            sum1[b] += r1[p0:p0 + h, j].sum()
            g, r = j // 32, j % 32
            sr, sc = 32 * (r % 4), 64 * (r // 4)
            sum2[b] += st[sr + i, 512 * g + sc:512 * g + sc + 64].sum()

    l1 = (-sum1 / olf).astype(np.float32)
    l2 = (sum2 / olf).astype(np.float32)
    return (l1, l2)


# revision 7
# speedup vs baseline: 1.1806x; 1.0165x over previous
"""GuidedAttentionLoss on 8 TRN2 cores — transposed-layout pipeline using
all five engines.

Layout: partition dim = input tokens y, free dim = output frames x
(att_ws transposed per sample). Work is packed into 128-partition "units"
(stacks of (sample, y-block, x-piece) items, first-fit-decreasing on
height), striped round-robin across the 8 cores with shared program
geometry (per-stripe widths = max over cores, emission ordered small ->
big -> small for fast pipeline fill/drain).

Per unit (L = padded x-width, L <= 1536):
  PE:   arg[p,x] = A_p*x + C_p*(x^2/4096) + bias_p  (K=3 fp16 matmul
        against a shared [x, x^2/4096, 1] ramp; the per-partition coeffs
        fold the full guided-attention quadratic -3.125*(y/il - x/ol)^2
        including x-piece offsets; bias rides as a third contraction row)
  Act:  e = Exp(arg)                                 (PSUM -> SBUF bf16)
  DVE:  t = (e - 1)*a   scalar_tensor_tensor, accum_out -> r1[:, unit]
  Pool: q = t*t         (gpsimd tensor_tensor, bf16)
  PE:   strip[slot] += ind.T @ q   (64-wide accumulating indicator
        matmuls -> per-item partial column sums in a PSUM strip bank)
Strips are staged to SBUF (DVE copy) and DMA'd out in 128-col blocks as
they complete; the host does the final tiny per-item gather in f64.
The coeff/ramp aux load is split so PE can start ~4.3us in; the Act
exp-table load is warmed by a dummy activation during the DMA fill, and
the PE clock is pre-ramped to full p-state by dummy matmuls into the
strip bank (overwritten by its memset) so the first real args run at
2.4 GHz instead of mid-clock.

Input att_ws is cast to bf16 on the host (rel tol 2e-2; measured total
error ~6e-4). HW exec time (CoreSim cost model, same inputs): 27318 ns
vs 81416 ns for the previous width-class row-layout kernel (2.98x).
"""
import numpy as np
import ml_dtypes

N_CORES = 8
SIGMA = 0.4
KC = 1.0 / (2 * SIGMA * SIGMA)  # 3.125
LMAX = 1536        # x-piece cap (arg psum tile: 3 banks)
MCAP = 8           # items per unit cap (ind columns / strip slot rows)
XMAX = 2048
LQUANT = 32

_cache = {}


# --------------------------------------------------------------- packing ---
def _pack(il, ol):
    """Build global units. Returns list of units; each unit:
    (L, items) with items = list of (b, k, xoff, xlen, h, p0)."""
    items = []
    B = len(il)
    for b in range(B):
        nb = (int(il[b]) + 127) // 128
        for k in range(nb):
            h = min(int(il[b]) - 128 * k, 128)
            xoff = 0
            while xoff < int(ol[b]):
                xlen = min(LMAX, int(ol[b]) - xoff)
                items.append((b, k, xoff, xlen, h))
                xoff += xlen
    # first-fit-decreasing on partition height, items in xlen-desc order so
    # a unit's L (max xlen = first item's xlen) never grows when stacking
    items.sort(key=lambda t: -t[3])
    units = []       # list of item lists
    room = []        # partition rows left per unit
    for it in items:
        h = it[4]
        for i, r in enumerate(room):
            if r >= h and len(units[i]) < MCAP:
                units[i].append(it)
                room[i] -= h
                break
        else:
            units.append([it])
            room.append(128 - h)
    out = []
    for u in units:
        L = -(-max(it[3] for it in u) // LQUANT) * LQUANT
        p0 = 0
        its = []
        for (b, k, xoff, xlen, h) in u:
            its.append((b, k, xoff, xlen, h, p0))
            p0 += h
        out.append((L, its))
    # sort units by L desc for striping
    out.sort(key=lambda t: -t[0])
    return out


def _stripe(units):
    """Assign global unit j -> (core j%8, stripe rank j//8), then permute
    stripe emission order: a couple of small stripes first (fast pipeline
    fill), the big ones in the middle, small again at the end (fast drain).
    Returns (U, widths_in_emission_order, ranks) with ranks[slot] = stripe
    rank (desc-width index) emitted at that slot."""
    U = (len(units) + N_CORES - 1) // N_CORES
    desc = [units[s * N_CORES][0] for s in range(U)]  # desc widths
    order = []
    if U > 3:
        order = [U - 1, U - 3] + list(range(U - 3)) + [U - 2]
    else:
        order = list(range(U))
    widths = [desc[r] for r in order]
    return U, widths, tuple(order)


def _engine_plan(widths):
    """Per stripe: 'v' (DVE) or 'g' (Pool) for the q = t*t pass, balancing
    measured busy rates (Pool TT costs 0.833 ns/col in the cost model; DVE
    TT 0.52 but DVE already runs every stt at 1.04)."""
    dve = sum(w * 1.0417 for w in widths) + 1300.0  # stt passes + copies
    pool = 0.0
    plan = []
    for w in widths:
        if dve + w * 0.5208 + 400.0 < pool + w * 0.8333 + 95.0:
            plan.append("v")
            dve += w * 0.5208
        else:
            plan.append("g")
            pool += w * 0.8333 + 95.0
    return tuple(plan)


# --------------------------------------------------------------- program ---
def _build_program(widths, plan, nchunks_dma):
    import concourse.bacc as bacc
    import concourse.mybir as mybir
    import concourse.tile as tile

    F32 = mybir.dt.float32
    BF16 = mybir.dt.bfloat16
    FP16 = mybir.dt.float16
    Exp = mybir.ActivationFunctionType.Exp
    Copy = mybir.ActivationFunctionType.Copy
    sub = mybir.AluOpType.subtract
    mult = mybir.AluOpType.mult

    U = len(widths)
    F = sum(widths)
    G = (U + 31) // 32  # strip generations

    # group stripes into DMA chunks: small first chunks so compute starts
    # early, growing to ~6K cols (12KB/partition, under the 64KB desc cap)
    chunks = []  # (u_start, u_end, col_off, width)
    u0, c0 = 0, 0
    targets = [512, 1536, 3072]  # then 6144
    for u in range(U + 1):
        w = sum(widths[u0:u])
        tgt = targets[len(chunks)] if len(chunks) < len(targets) else 6144
        if u == U or (w >= tgt and u > u0):
            if u > u0:
                chunks.append((u0, u, c0, w))
                c0 += w
                u0 = u

    nc = bacc.Bacc("TRN2", target_bir_lowering=False, debug=False,
                   num_devices=1)
    A = nc.declare_dram_parameter("A", [128, F], BF16, isOutput=False)
    # RC = [x-ramp rows (x, x^2/4096, 1) | per-unit (A,C,bias) coeff
    # columns], one fp16 aux load; bias rides the matmul as a K=3 row
    RCA = nc.declare_dram_parameter("RCA", [3, 128 * U + 512], FP16,
                                    isOutput=False)
    RCB = nc.declare_dram_parameter("RCB", [3, XMAX - 512], FP16,
                                    isOutput=False)
    IND = nc.declare_dram_parameter("IND", [128, MCAP * U], BF16,
                                    isOutput=False)
    R1 = nc.declare_dram_parameter("R1", [128, U], F32, isOutput=True)
    STR = nc.declare_dram_parameter("STR", [128, 512 * G], F32, isOutput=True)

    with tile.TileContext(nc) as tc:
        with tc.tile_pool(name="aux", bufs=1) as aux, \
             tc.tile_pool(name="pa", bufs=1) as pa, \
             tc.tile_pool(name="pe_", bufs=4) as pev, \
             tc.tile_pool(name="pt", bufs=4) as pt, \
             tc.tile_pool(name="pq", bufs=4) as pq, \
             tc.tile_pool(name="psarg", bufs=2, space="PSUM") as psarg, \
             tc.tile_pool(name="psstr", bufs=2, space="PSUM") as psstr:
            # issue order matters: rc (PE needs it first), then the first
            # small a-chunk (stt), then bias/ind, then remaining chunks
            warm = aux.tile([128, 1], F32)
            nc.vector.memset(warm[:], 0.0)
            warm2 = aux.tile([128, 1], F32)
            nc.scalar.activation(warm2[:], warm[:], Exp, scale=1.0)
            # PE p-state warm-up: keep the tensor engine busy from t~1.5us
            # so the first real args run at full clock. Dummies write the
            # gen-0 strip bank, which is memset afterwards anyway.
            wpe = aux.tile([2, 512], FP16)
            nc.vector.memset(wpe[:], 0.0)
            strip0 = psstr.tile([128, 512], F32, tag="strip")
            for _ in range(5):
                nc.tensor.matmul(strip0[:, :512], wpe[:, :128],
                                 wpe[:, :512], start=True, stop=True)
            rca = aux.tile([3, 128 * U + 512], FP16)
            nc.sync.dma_start(rca[:], RCA[:])
            rcb = aux.tile([3, XMAX - 512], FP16)
            nc.scalar.dma_start(rcb[:], RCB[:])

            def ramp_sl(c, cw):
                if c + cw <= 512:
                    return rca[:, 128 * U + c:128 * U + c + cw]
                return rcb[:, c - 512:c - 512 + cw]
            ind = aux.tile([128, MCAP * U], BF16)
            r1 = aux.tile([128, U], F32)

            # chunked input loads (separate tiles so compute can start early)
            atiles = []
            for ci, (us, ue, coff, w) in enumerate(chunks):
                t_ = pa.tile([128, w], BF16, tag=f"a{us}")
                nc.sync.dma_start(t_[:], A[:, coff:coff + w])
                atiles.append((us, ue, coff, t_))
                if ci == 0:
                    nc.sync.dma_start(ind[:], IND[:])

            def a_slice(u, off_u, L):
                for (us, ue, coff, t_) in atiles:
                    if us <= u < ue:
                        return t_[:, off_u - coff:off_u - coff + L]
                raise AssertionError

            strip = None
            stage = None
            off = 0
            # xoff per stripe is geometry: ramp slice start. xoff is the
            # SAME for all items in a unit? No - items have different xoff.
            # arg uses absolute x: ramp slice must match each item's x range.
            # => all items in a unit must share the same x-window [xw, xw+L).
            # Handled in packing: xoff passed via widths-aligned plan...
            # (see _pack2: units carry xw; here xw comes in via plan tuple)
            # pair consecutive <=768-wide units: both args share one psum
            # tile (B at column 768) and a single Act Exp covers both
            pair_first = {}
            pair_second = {}
            # (pairing disabled: measured slower than per-unit Act)

            et_pair = None
            for u in range(U):
                L = widths[u]
                g, r = u // 32, u % 32
                sr, sc = 32 * (r % 4), 64 * (r // 4)
                if r == 0:
                    strip = strip0 if g == 0 else \
                        psstr.tile([128, 512], F32, tag="strip")
                    nc.vector.memset(strip[:], 0.0)
                if u in pair_second:
                    et = et_pair[:, 768:768 + L]
                elif u in pair_first:
                    ub = pair_first[u]
                    lb = widths[ub]
                    arg = psarg.tile([128, LMAX], F32, tag="arg")
                    # A's chunks extended to the full 768 cols: the gap
                    # [L_A, 768) computes real arg values (always <= 0, so
                    # exp <= 1) that nothing reads — keeps psum initialized
                    for uu, base, ext in ((u, 0, 768), (ub, 768, lb)):
                        for c in range(0, ext, 512):
                            cw = min(512, ext - c)
                            bb = base + c
                            cw = min(cw, 512 - (bb % 512)) if bb % 512 \
                                else cw
                            nc.tensor.matmul(
                                arg[:, bb:bb + cw],
                                rc[:, XMAX + 128 * uu:XMAX + 128 * uu + 128],
                                ramp[:, c:c + cw],
                                start=True, stop=True)
                            if cw < min(512, ext - c):
                                c2 = c + cw
                                cw2 = min(512, ext - c) - cw
                                nc.tensor.matmul(
                                    arg[:, base + c2:base + c2 + cw2],
                                    rc[:, XMAX + 128 * uu:
                                        XMAX + 128 * uu + 128],
                                    ramp[:, c2:c2 + cw2],
                                    start=True, stop=True)
                    et_pair = pev.tile([128, 768 + lb], BF16, tag="e")
                    nc.scalar.activation(et_pair[:], arg[:, :768 + lb],
                                         Exp, scale=1.0)
                    et = et_pair[:, :L]
                else:
                    arg = psarg.tile([128, LMAX], F32, tag="arg")
                    for c in range(0, L, 512):
                        cw = min(512, L - c)
                        nc.tensor.matmul(
                            arg[:, c:c + cw],
                            rca[:, 128 * u:128 * u + 128],
                            ramp_sl(c, cw),
                            start=True, stop=True)
                    et_full = pev.tile([128, L], BF16, tag="e")
                    nc.scalar.activation(et_full[:], arg[:, :L], Exp,
                                         scale=1.0)
                    et = et_full[:]
                tt = pt.tile([128, L], BF16, tag="t")
                nc.vector.scalar_tensor_tensor(
                    tt[:], et[:], 1.0, a_slice(u, off, L), sub, mult,
                    accum_out=r1[:, u:u + 1])
                qt = pq.tile([128, L], BF16, tag="q")
                qeng = nc.vector if plan[u] == "v" else nc.gpsimd
                qeng.tensor_tensor(qt[:], tt[:], tt[:], mult)
                nch = (L + 63) // 64
                for ci, c in enumerate(range(0, L, 64)):
                    cw = min(64, L - c)
                    nc.tensor.matmul(
                        strip[sr:sr + MCAP, sc:sc + cw],
                        ind[:, MCAP * u:MCAP * u + MCAP],
                        qt[:, c:c + cw],
                        start=(ci == 0), stop=(ci == nch - 1),
                        tile_position=(0, sr))
                last = u == U - 1
                if (r % 8 == 7 and not last) or (last and r % 8 != 7):
                    # flush completed 128-col block(s) of this generation
                    lo = 128 * (r // 8)
                    hi = lo + 128 if not last else 64 * (r // 4) + 64
                    sh = pev.tile([128, 512], F32, tag="stg")
                    nc.vector.tensor_copy(sh[:, lo:hi], strip[:, lo:hi])
                    nc.sync.dma_start(STR[:, 512 * g + lo:512 * g + hi],
                                      sh[:, lo:hi])
                elif last:
                    lo = 128 * (r // 8)
                    hi = 64 * (r // 4) + 64
                    sh = pev.tile([128, 512], F32, tag="stg")
                    nc.vector.tensor_copy(sh[:, lo:hi], strip[:, lo:hi])
                    nc.sync.dma_start(STR[:, 512 * g + lo:512 * g + hi],
                                      sh[:, lo:hi])
                if u == (3 * U) // 4:
                    usplit = u + 1
                    nc.sync.dma_start(R1[:, :usplit], r1[:, :usplit])
                off += L

            nc.scalar.dma_start(R1[:, usplit:], r1[:, usplit:])
    nc.compile()
    return nc


# ---------------------------------------------------------------- kernel ---
def kernel(att_ws, ilens, olens, _trace=False, _tracedir=None):
    from concourse.bass_utils import run_bass_kernel_spmd

    att_ws = np.asarray(att_ws)
    il = np.asarray(ilens).astype(np.int64)
    ol = np.asarray(olens).astype(np.int64)
    B, T_out, T_in = att_ws.shape

    units = _pack(il, ol)
    U, widths, ranks = _stripe(units)
    plan = _engine_plan(widths)
    G = (U + 31) // 32
    F = sum(widths)

    # x-window consistency: items within a global unit may have different
    # xoff. The program indexes ramp[:, 0:L] (x starting at 0). For items
    # with xoff > 0 we instead fold the shift into per-partition constants:
    # arg(x') = A_p*(x'+xoff) + C_p*(x'+xoff)^2/..  -- quadratic in x' with
    # modified A'_p, C'_p (C unchanged), bias absorbs constant:
    #   A'_p = A_p + 2*xoff*Cq_p ; bias' += A_p*xoff + Cq_p*xoff^2
    # where Cq_p = -KC/ol^2 (true x^2 coeff; C_p = 4096*Cq_p).
    ilf = il.astype(np.float64)
    olf = ol.astype(np.float64)

    offs = np.concatenate([[0], np.cumsum(widths)]).astype(np.int64)
    in_maps = []
    meta = []  # per core: list of (j, item list)
    x = np.arange(XMAX, dtype=np.float64)
    RAMP = np.stack([x, x * x / 4096.0, np.ones(XMAX)])

    for c in range(N_CORES):
        Ar = np.zeros((128, F), np.float32)
        AC = np.zeros((3, 128 * U), np.float32)
        IND = np.zeros((128, MCAP * U), np.float32)
        mymeta = []
        for j in range(U):
            gidx = ranks[j] * N_CORES + c
            if gidx >= len(units):
                continue
            L, its = units[gidx]
            assert L <= widths[j]
            for i, (b, k, xoff, xlen, h, p0) in enumerate(its):
                yv = (128 * k + np.arange(h)).astype(np.float64)
                z = yv / ilf[b]
                Aq = 2.0 * KC * z / olf[b]          # x coeff
                Cq = -KC / (olf[b] ** 2)            # x^2 coeff
                Ap = Aq + 2.0 * xoff * Cq
                biasp = -KC * z * z + Aq * xoff + Cq * xoff * xoff
                AC[0, 128 * j + p0:128 * j + p0 + h] = Ap
                AC[1, 128 * j + p0:128 * j + p0 + h] = 4096.0 * Cq
                AC[2, 128 * j + p0:128 * j + p0 + h] = biasp
                IND[p0:p0 + h, MCAP * j + i] = 1.0
                Ar[p0:p0 + h, offs[j]:offs[j] + xlen] = \
                    att_ws[b, xoff:xoff + xlen, 128 * k:128 * k + h].T
                mymeta.append((j, i, b, p0, h))
        in_maps.append({
            "A": Ar.astype(ml_dtypes.bfloat16),
            "RCA": np.concatenate([AC, RAMP[:, :512]],
                                  axis=1).astype(np.float16),
            "RCB": RAMP[:, 512:].astype(np.float16),
            "IND": IND.astype(ml_dtypes.bfloat16),
        })
        meta.append(mymeta)

    key = (tuple(widths), plan)
    if key not in _cache:
        _cache[key] = _build_program(list(widths), plan, 8)
    nc = _cache[key]
    kw = {}
    if _trace:
        kw = dict(trace=True, tmpdir=_tracedir)
    res = run_bass_kernel_spmd(nc, in_maps, list(range(N_CORES)), **kw)

    sum1 = np.zeros(B, np.float64)
    sum2 = np.zeros(B, np.float64)
    for c in range(N_CORES):
        r1 = np.asarray(res.results[c]["R1"], np.float64)
        st = np.asarray(res.results[c]["STR"], np.float64)
        for (j, i, b, p0, h) in meta[c]:
            sum1[b] += r1[p0:p0 + h, j].sum()
            g, r = j // 32, j % 32
            sr, sc = 32 * (r % 4), 64 * (r // 4)
            sum2[b] += st[sr + i, 512 * g + sc:512 * g + sc + 64].sum()

    l1 = (-sum1 / olf).astype(np.float32)
    l2 = (sum2 / olf).astype(np.float32)
    return (l1, l2)
